# revision 4
# baseline (speedup 1.0000x reference)
"""GCN (3-layer EnergyFlowGNN) Trainium2 Bass kernel, 8-core SPMD.

Strategy: node-sharded pull design. Core c owns dst nodes [c*NPC, (c+1)*NPC).
Symmetric norm factorizes: out = dis_d * sum_e dis_s * T[src_e]; dis_src is
folded into the PE selection matrix, dis_dst applied on the PSUM drain.
Per dst-window (128 nodes) edges are grouped into 4 src-chunks (table views
of <=25000 rows so dma_gather's int16 indices reach them), gathered compactly
with Q7 dma_gather, then scatter-added on the TensorEngine via a weighted
one-hot selection matmul accumulating in PSUM. Layer tables are exchanged
with AllGather. Layer 3 gathers 64-packed scalars and extracts via a second
one-hot + reduce.
"""
import sys, os
sys.path.insert(0, "/opt/trn_rl_repo")
import numpy as np

import concourse.bacc as bacc
import concourse.mybir as mybir
import concourse.tile as tile
from concourse.tile import add_dep_helper
from concourse.bass_utils import run_bass_kernel_spmd

N_NODES = int(os.environ.get("KN", "100000"))
N_EDGES = 3200000
NF = 5
H = 64
NCORES = 8
NPC = N_NODES // NCORES          # nodes per core
NCHUNK = 4
CHUNK = N_NODES // NCHUNK        # table rows per chunk view (int16-safe)
PASSW = int(os.environ.get("KPASSW", "4"))   # dst windows per gather pass

_prog_cache = {}


def _round128(x):
    return (x + 127) & ~127


def _prep(x, edge_index, W3):
    """Host-side sharding/layout. Returns per-core input maps + static meta."""
    src = np.concatenate([edge_index[0], np.arange(N_NODES, dtype=np.int64)])
    dst = np.concatenate([edge_index[1], np.arange(N_NODES, dtype=np.int64)])
    deg = np.bincount(dst, minlength=N_NODES).astype(np.float64)
    dis = np.where(deg > 0, 1.0 / np.sqrt(deg), 0.0).astype(np.float32)

    core = dst // NPC
    wloc = (dst - core * NPC) // 128
    g = src // CHUNK
    NW = (NPC + 127) // 128

    # order edges by (core, window, chunk)
    order = np.lexsort((g, wloc, core))
    src_o, dst_o, core_o, w_o, g_o = (a[order] for a in (src, dst, core, wloc, g))

    # segment counts per (core, w, g)
    seg_cnt = np.zeros((NCORES, NW, NCHUNK), np.int64)
    np.add.at(seg_cnt, (core_o, w_o, g_o), 1)
    SEG = _round128(seg_cnt.max(axis=0))          # [NW, NCHUNK] common sizes
    Tg = SEG.sum(axis=0)                          # slots per chunk stream

    # per-core chunk streams
    idx2 = np.zeros((NCORES, NCHUNK, Tg.max()), np.int16)
    dstrel = np.full((NCORES, NCHUNK, Tg.max()), 128.0, np.float32)
    vsrc = np.zeros((NCORES, NCHUNK, Tg.max()), np.float32)

    # slice boundaries of each core's sorted edges per (w, g)
    base = np.searchsorted(core_o, np.arange(NCORES))
    end = np.searchsorted(core_o, np.arange(NCORES), side="right")
    segoff = np.zeros(NCHUNK, np.int64)
    seg_starts = np.zeros((NW, NCHUNK), np.int64)
    for w in range(NW):
        for gg in range(NCHUNK):
            seg_starts[w, gg] = segoff[gg]
            segoff[gg] += SEG[w, gg]
    for c in range(NCORES):
        s_c = src_o[base[c]:end[c]]
        d_c = dst_o[base[c]:end[c]]
        w_c = w_o[base[c]:end[c]]
        g_c = g_o[base[c]:end[c]]
        key = w_c * NCHUNK + g_c
        bounds = np.searchsorted(key, np.arange(NW * NCHUNK + 1))
        for w in range(NW):
            for gg in range(NCHUNK):
                lo, hi = bounds[w * NCHUNK + gg], bounds[w * NCHUNK + gg + 1]
                n = hi - lo
                o = seg_starts[w, gg]
                idx2[c, gg, o:o + n] = (s_c[lo:hi] - gg * CHUNK).astype(np.int16)
                dstrel[c, gg, o:o + n] = ((d_c[lo:hi] - c * NPC) % 128).astype(np.float32)
                vsrc[c, gg, o:o + n] = dis[s_c[lo:hi]]

    # device layouts
    def wrap16(a):  # [T] -> [128, T//16] (idx i at partition i%16, col i//16)
        t = a.reshape(-1, 16).T
        return np.tile(t, (8, 1)).copy()

    def colmaj(a):  # [T] -> [128, T//128] (col per 128-group)
        return np.ascontiguousarray(a.reshape(-1, 128).T)

    xpad = np.zeros((N_NODES, H), np.float32)
    xpad[:, :NF] = x * dis[:, None]
    iota = np.tile(np.arange(128, dtype=np.float32)[None, :], (128, 1))
    ident = np.eye(128, dtype=np.float32)

    in_maps = []
    for c in range(NCORES):
        m = {"xpad": xpad, "iota": iota, "ident": ident}
        for gg in range(NCHUNK):
            m[f"idx2_{gg}"] = wrap16(idx2[c, gg, :Tg[gg]])
            m[f"dstrel_{gg}"] = colmaj(dstrel[c, gg, :Tg[gg]])
        dd = np.zeros(NW * 128, np.float32)
        dd[:NPC] = dis[c * NPC:(c + 1) * NPC]
        m["disd"] = colmaj(dd)
        m["disdr"] = dd.reshape(1, NW * 128)
        in_maps.append(m)
    meta = (tuple(map(tuple, SEG)), tuple(Tg))
    return in_maps, meta


def _build(meta):
    SEG = np.array(meta[0])        # [NW, NCHUNK]
    Tg = list(meta[1])
    NW = SEG.shape[0]
    f32 = mybir.dt.float32
    nc = bacc.Bacc("TRN2", target_bir_lowering=False, debug=False,
                   num_devices=NCORES)

    xpad = nc.dram_tensor("xpad", [N_NODES, H], f32, kind="ExternalInput")
    iota_in = nc.dram_tensor("iota", [128, 128], f32, kind="ExternalInput")
    id_in = nc.dram_tensor("ident", [128, 128], f32, kind="ExternalInput")
    W1_in = nc.dram_tensor("W1", [NF, H], f32, kind="ExternalInput")
    b1_in = nc.dram_tensor("b1c", [H, 1], f32, kind="ExternalInput")
    W2_in = nc.dram_tensor("W2", [H, H], f32, kind="ExternalInput")
    b2_in = nc.dram_tensor("b2c", [H, 1], f32, kind="ExternalInput")
    W3_in = nc.dram_tensor("W3", [H, 1], f32, kind="ExternalInput")
    b3_in = nc.dram_tensor("b3r", [128, 1], f32, kind="ExternalInput")
    disd_in = nc.dram_tensor("disd", [128, NW], f32, kind="ExternalInput")
    disdr_in = nc.dram_tensor("disdr", [1, NW * 128], f32, kind="ExternalInput")
    ins_g = {}
    for gg in range(NCHUNK):
        ins_g[("i2", gg)] = nc.dram_tensor(f"idx2_{gg}", [128, Tg[gg] // 16],
                                           mybir.dt.int16, kind="ExternalInput")
        for nm in ("dstrel",):
            ins_g[(nm, gg)] = nc.dram_tensor(f"{nm}_{gg}", [128, Tg[gg] // 128],
                                             f32, kind="ExternalInput")
    out = nc.dram_tensor("out", [NPC, 1], f32, kind="ExternalOutput")
    h1loc = nc.dram_tensor("h1loc", [NPC, H], f32)
    s2rep = nc.dram_tensor("s2rep", [NPC, H], f32)
    T2 = nc.dram_tensor("T2", [N_NODES, H], f32, addr_space="Shared")
    T3 = nc.dram_tensor("T3", [N_NODES, H], f32, addr_space="Shared")

    groups_per_pass = []   # per pass: list of (w, gg, buf_col_offset)
    NPASS = (NW + PASSW - 1) // PASSW

    from contextlib import ExitStack
    _gstk = ExitStack()
    with tile.TileContext(nc) as tc:
        cpool = _gstk.enter_context(tc.tile_pool(name="const", bufs=1))
        iota_t = cpool.tile([128, 128], f32); nc.sync.dma_start(out=iota_t[:], in_=iota_in[:])
        id_t = cpool.tile([128, 128], f32); nc.sync.dma_start(out=id_t[:], in_=id_in[:])
        W1_t = cpool.tile([NF, H], f32); nc.sync.dma_start(out=W1_t[:], in_=W1_in[:])
        b1_t = cpool.tile([H, 1], f32); nc.sync.dma_start(out=b1_t[:], in_=b1_in[:])
        W2_t = cpool.tile([H, H], f32); nc.sync.dma_start(out=W2_t[:], in_=W2_in[:])
        b2_t = cpool.tile([H, 1], f32); nc.sync.dma_start(out=b2_t[:], in_=b2_in[:])
        W3_t = cpool.tile([H, 1], f32); nc.sync.dma_start(out=W3_t[:], in_=W3_in[:])
        b3_t = cpool.tile([128, 1], f32); nc.sync.dma_start(out=b3_t[:], in_=b3_in[:])
        disd_t = cpool.tile([128, NW], f32); nc.sync.dma_start(out=disd_t[:], in_=disd_in[:])
        disdr_t = cpool.tile([1, NW * 128], f32); nc.sync.dma_start(out=disdr_t[:], in_=disdr_in[:])
        meta_t = {}
        for gg in range(NCHUNK):
            for nm in ("dstrel",):
                t = cpool.tile([128, Tg[gg] // 128], f32, tag=f"{nm}{gg}")
                nc.sync.dma_start(out=t[:], in_=ins_g[(nm, gg)][:])
                meta_t[(nm, gg)] = t
        ones64_t = cpool.tile([128, H], f32)
        nc.vector.memset(ones64_t[:], 1.0)

        all_gathers = []

        def run_layer(layer, table_views, idx_key, table_dep=None):
            """layer in (1,2,3). table_views: per-chunk DRAM APs.
            Returns the output-write DMA instructions."""
            from contextlib import ExitStack
            stk = ExitStack()
            mpool = stk.enter_context(tc.tile_pool(name=f"msg{layer}", bufs=2))
            ppool = stk.enter_context(tc.tile_pool(name=f"ps{layer}", bufs=PASSW, space="PSUM"))
            gpool = stk.enter_context(tc.tile_pool(name=f"gm{layer}", bufs=1, space="PSUM"))
            spool = stk.enter_context(tc.tile_pool(name=f"sb{layer}", bufs=3))
            m2pool = stk.enter_context(tc.tile_pool(name=f"m2{layer}", bufs=4))
            wdmas = []
            C_out = 1 if layer == 3 else H
            seg_starts = np.zeros(NCHUNK, np.int64)
            seg_off = np.zeros((NW, NCHUNK), np.int64)
            for w in range(NW):
                for gg in range(NCHUNK):
                    seg_off[w, gg] = seg_starts[gg]
                    seg_starts[gg] += SEG[w, gg]
            for p in range(NPASS):
                ws = range(p * PASSW, min((p + 1) * PASSW, NW))
                bufs, offs = {}, {}
                for gg in range(NCHUNK):
                    n = int(SEG[list(ws), gg].sum())
                    if n == 0:
                        continue
                    c0 = int(seg_off[list(ws)[0], gg])
                    it = mpool.tile([128, max(n, 128) // 16], mybir.dt.int16,
                                    tag=f"it{gg}")
                    ld = nc.sync.dma_start(
                        out=it[:, :n // 16],
                        in_=ins_g[(idx_key, gg)][:, c0 // 16:(c0 + n) // 16])
                    mt = mpool.tile([128, (n // 128) * H], f32, tag=f"mt{gg}")
                    gv = mt[:].rearrange("p (k c) -> p k c", k=n // 128, c=H)
                    g = nc.gpsimd.dma_gather(
                        out_ap=gv, in_ap=table_views[gg], idxs_ap=it[:, :n // 16],
                        num_idxs=n, num_idxs_reg=n, elem_size=H,
                        single_packet=False)
                    add_dep_helper(g.ins, ld.ins, True, "gather reads idx")
                    if table_dep is not None:
                        add_dep_helper(g.ins, table_dep.ins, True, "gather reads table")
                    all_gathers.append(g)
                    bufs[gg] = (mt, g)
                    offs[gg] = c0
                for w in ws:
                    acc = ppool.tile([128, C_out], f32, tag="acc")
                    ngrp = int(SEG[w].sum()) // 128
                    gi = 0
                    for gg in range(NCHUNK):
                        nseg = int(SEG[w, gg])
                        if nseg == 0:
                            continue
                        mt, g = bufs[gg]
                        local0 = int(seg_off[w, gg]) - offs[gg]
                        for k in range(nseg // 128):
                            col = (int(seg_off[w, gg]) + k * 128) // 128
                            dre = meta_t[("dstrel", gg)][:, col:col + 1]
                            m2 = m2pool.tile([128, 128], f32, tag="m2")
                            nc.vector.tensor_scalar(
                                out=m2[:], in0=iota_t[:], scalar1=dre,
                                scalar2=None, op0=mybir.AluOpType.is_equal)
                            kk = local0 // 128 + k
                            if layer == 3:
                                rhs = mt[:, kk * H:kk * H + 1]
                            else:
                                rhs = mt[:, kk * H:(kk + 1) * H]
                            mm = nc.tensor.matmul(
                                out=acc[:], lhsT=m2[:], rhs=rhs,
                                start=(gi == 0), stop=(gi == ngrp - 1))
                            add_dep_helper(mm.ins, g.ins, True, "mm reads msg")
                            gi += 1
                    # drain: scale by dis_d
                    wn = min(128, NPC - w * 128)
                    ags = spool.tile([128, C_out], f32, tag="ags")
                    nc.vector.tensor_scalar_mul(ags[:], acc[:],
                                                disd_t[:, w:w + 1])
                    if layer == 1:
                        tp = gpool.tile([NF, 128], f32, tag="tp")
                        nc.tensor.transpose(out=tp[:], in_=ags[:, :NF],
                                            identity=id_t[:])
                        tps = spool.tile([NF, 128], f32, tag="tps")
                        nc.vector.tensor_copy(out=tps[:], in_=tp[:])
                        hT = gpool.tile([H, 128], f32, tag="hT")
                        nc.tensor.matmul(out=hT[:], lhsT=W1_t[:], rhs=tps[:],
                                         start=True, stop=True)
                        hTs = spool.tile([H, 128], f32, tag="hTs")
                        nc.scalar.activation(hTs[:], hT[:],
                                             mybir.ActivationFunctionType.Relu,
                                             bias=b1_t[:])
                        hb = gpool.tile([128, H], f32, tag="hb")
                        nc.tensor.transpose(out=hb[:], in_=hTs[:],
                                            identity=id_t[:H, :H])
                        hbs = spool.tile([128, H], f32, tag="hbs")
                        nc.vector.tensor_scalar_mul(hbs[:], hb[:],
                                                    disd_t[:, w:w + 1])
                        wdmas.append(nc.sync.dma_start(
                            out=h1loc[w * 128:w * 128 + wn, :], in_=hbs[:wn, :]))
                    elif layer == 2:
                        tp = gpool.tile([H, 128], f32, tag="tp")
                        nc.tensor.transpose(out=tp[:], in_=ags[:],
                                            identity=id_t[:])
                        tps = spool.tile([H, 128], f32, tag="tps")
                        nc.vector.tensor_copy(out=tps[:], in_=tp[:])
                        hT = gpool.tile([H, 128], f32, tag="hT")
                        nc.tensor.matmul(out=hT[:], lhsT=W2_t[:], rhs=tps[:],
                                         start=True, stop=True)
                        hTs = spool.tile([H, 128], f32, tag="hTs")
                        nc.scalar.activation(hTs[:], hT[:],
                                             mybir.ActivationFunctionType.Relu,
                                             bias=b2_t[:])
                        s2p = gpool.tile([1, 128], f32, tag="s2p")
                        nc.tensor.matmul(out=s2p[:], lhsT=W3_t[:], rhs=hTs[:],
                                         start=True, stop=True)
                        s2s = spool.tile([1, 128], f32, tag="s2s")
                        nc.vector.tensor_tensor(
                            out=s2s[:], in0=s2p[:],
                            in1=disdr_t[:, w * 128:w * 128 + 128],
                            op=mybir.AluOpType.mult)
                        s2dT = gpool.tile([128, 1], f32, tag="s2dT")
                        nc.tensor.transpose(out=s2dT[:], in_=s2s[:],
                                            identity=id_t[:1, :1])
                        rep = spool.tile([128, H], f32, tag="rep")
                        nc.vector.tensor_scalar_mul(rep[:], ones64_t[:],
                                                    s2dT[:])
                        wdmas.append(nc.sync.dma_start(
                            out=s2rep[w * 128:w * 128 + wn, :], in_=rep[:wn, :]))
                    else:
                        o = spool.tile([128, 1], f32, tag="o3")
                        nc.vector.tensor_scalar(
                            out=o[:], in0=acc[:], scalar1=disd_t[:, w:w + 1],
                            scalar2=b3_t[:], op0=mybir.AluOpType.mult,
                            op1=mybir.AluOpType.add)
                        wdmas.append(nc.sync.dma_start(
                            out=out[w * 128:w * 128 + wn, :], in_=o[:wn, :]))
            stk.close()
            return wdmas

        # ---- layer 1: tables are xpad chunk views
        tv1 = [xpad[gg * CHUNK:(gg + 1) * CHUNK, :] for gg in range(NCHUNK)]
        wd1 = run_layer(1, tv1, "i2")
        # ---- allgather h1 -> T2
        coll1 = nc.gpsimd.collective_compute(
            "AllGather", mybir.AluOpType.bypass,
            replica_groups=[list(range(NCORES))],
            ins=[h1loc[:, :]], outs=[T2[:, :]])
        for d in wd1:
            add_dep_helper(coll1.ins, d.ins, True, "allgather waits h1 writes")
        tv2 = [T2[gg * CHUNK:(gg + 1) * CHUNK, :] for gg in range(NCHUNK)]
        wd2 = run_layer(2, tv2, "i2", table_dep=coll1)
        # allgather replicated s2 rows -> T3 [N, 64]
        coll2 = nc.gpsimd.collective_compute(
            "AllGather", mybir.AluOpType.bypass,
            replica_groups=[list(range(NCORES))],
            ins=[s2rep[:, :]], outs=[T3[:, :]])
        for d in wd2:
            add_dep_helper(coll2.ins, d.ins, True, "allgather waits s2 writes")
        tv3 = [T3[gg * CHUNK:(gg + 1) * CHUNK, :] for gg in range(NCHUNK)]
        run_layer(3, tv3, "i2", table_dep=coll2)

        # drain guard: pool engine must wait for outstanding gathers
        guard = cpool.tile([128, H], f32, tag="guard")
        pw = nc.gpsimd.dma_start(out=guard[:], in_=T2[:128, :])
        for g in all_gathers[-12:]:
            add_dep_helper(pw.ins, g.ins, True, "pool drain guard")
        _gstk.close()
    nc.compile()
    return nc


def kernel(x, edge_index, W1, b1, W2, b2, W3, b3):
    x = np.asarray(x, np.float32)
    edge_index = np.asarray(edge_index, np.int64)
    in_maps, meta = _prep(x, edge_index, W3)
    key = hash(meta)
    if key not in _prog_cache:
        _prog_cache[key] = _build(meta)
    nc = _prog_cache[key]
    shared = {
        "W1": np.asarray(W1, np.float32),
        "b1c": np.asarray(b1, np.float32).reshape(H, 1),
        "W2": np.asarray(W2, np.float32),
        "b2c": np.asarray(b2, np.float32).reshape(H, 1),
        "W3": np.asarray(W3, np.float32).reshape(H, 1),
        "b3r": np.full((128, 1), np.float32(np.asarray(b3).reshape(-1)[0])),
    }
    for m in in_maps:
        m.update(shared)
    res = run_bass_kernel_spmd(nc, in_maps, core_ids=list(range(NCORES)))
    outp = np.concatenate([res.results[c]["out"] for c in range(NCORES)], axis=0)
    return outp.astype(np.float32)

